# revision 15
# baseline (speedup 1.0000x reference)
"""Trainium2 Bass kernel for nn_AutoEncoder3D (chamfer-loss autoencoder).

Strategy (8 NeuronCores, SPMD with per-core data):
  core c -> batch b = c // 4, quarter q = c % 4 of generated points.
  Each core: full encoder (bf16 matmuls), decoder for its quarter of the
  3072 output columns, then fused cdist+min over the [16384, 1024] chamfer
  block using a lifted-embedding bf16 hi/lo matmul on the PE, ACT
  PSUM->SBUF fp16 copies, and DVE min folds.  Host combines per-core
  row-min sums and col-min partials.

v5 deltas vs v4 (262us traced):
  - K=15 lift instead of 20: psi rows [hi, hi, lo], phi rows
    [hi, lo, hi-dup] -- drops only the negligible lo*lo term and halves
    the phi dup/replica SBUF traffic.
  - preamble: ACT table load first (no ident dependency), no mm1-gating
    of decoder weights (wd3 rides the SP ring behind w1, chunk-gated
    d3p), w1 tail chunks split small, decoder matmuls padded to full
    128-partition height (2 col/cycle instead of quadrant-tiled
    1 col/cycle: d3p stream 8.3us -> ~4us).
  - phi pipeline: DRAM bounce gathers only the 10 unique rows; the hi
    dup (10:15) and the q64 replica (64:79) are cheap wide SBUF->SBUF
    DMAs per piece; pieces sized (2,6,16,20,20) grid cells so the
    distance loop starts ~10us earlier and is never gather-starved.
  - colrun kept as 2 lanes; each 16-tile batch tree-mins its bb block
    in place (same DVE element count, one extra instruction) so the
    epilogue fold collapses from ~4.6us to one 1024-wide min.
  - epilogue: single lane fold + transposes + one reduce; tail batches
    end at 8 tiles.
"""

import numpy as np
import ml_dtypes

import concourse.bass as bass
import concourse.mybir as mybir
import concourse.tile as tile_mod
from concourse.bass_utils import run_bass_kernel_spmd
from concourse.masks import make_identity
from concourse.tile import TileContext

F32 = mybir.dt.float32
F32R = mybir.dt.float32r
F16 = mybir.dt.float16
BF16 = mybir.dt.bfloat16
A = mybir.AluOpType
AFT = mybir.ActivationFunctionType
AX = mybir.AxisListType

B = 2
G = 64
M = 1024
NCORES = 8
JQ = 256          # generated points per grid cell handled per core
NLOC = G * JQ     # 16384 generated points per core
NT = NLOC // 128  # 128 n-tiles
K = 15            # lift rows: [psi_hi, psi_hi, psi_lo] x [phi_hi, phi_lo, phi_hi]

# w1 k-chunk splits (tail kept small so mm1 finishes right after the DMA)
W1CH = [(0, 5), (5, 5), (10, 5), (15, 5), (20, 3), (23, 2)]

# phi piece schedule (grid-cell ranges).  Tiles of piece p unblock as its
# gather (+dup/replica) DMAs land; consumption (~1.34us/tile) is far slower
# than the gather so only piece 0 gates the loop start.  The first two
# pieces gather every phi row region directly from the DRAM bounce in
# parallel (one hop after the bounce-out); the rest use cheap wide SBUF
# copies for the dup/replica rows.
OUTPIECES = [(0, 2), (2, 8), (8, 24), (24, 64)]
GPIECES = [(0, 2), (2, 8), (8, 24), (24, 44), (44, 64)]
NDIRECT = 2

# distance-phase batches, aligned to piece tile boundaries
# pieces cover tiles [0,4), [4,16), [16,48), [48,88), [88,128)
BATCHES = [(0, 2), (2, 2), (4, 4), (8, 8),
           (16, 16), (32, 16), (48, 16), (64, 16), (80, 8),
           (88, 16), (104, 16), (120, 8)]


# ---------------------------------------------------------------------------
# Tile-framework patches: this walrus build allows at most ONE sync wait per
# instruction.  (a) split multi-wait instructions with preceding no-ops,
# (b) replace the context-exit drain (which carries one wait per live proc)
# with individual SP wait_ge instructions and a single exit barrier.
# ---------------------------------------------------------------------------
if not getattr(tile_mod, "_ae3d_wait_patch", False):
    tile_mod._ae3d_wait_patch = True

    _orig_commit = tile_mod.TileContext._commit_instruction

    def _commit_split(self, inst, lazy_reg_writes=True):
        si = getattr(inst, "sync_info", None)
        if si is not None and si.on_wait and len(si.on_wait) > 1:
            waits = list(si.on_wait)
            for w in waits[:-1]:
                nop = mybir.InstNoOp(
                    name=self.nc.get_next_instruction_name(),
                    sync_info=mybir.SyncInfo(on_wait=[w], on_update=[]),
                    bass_nofuse=True,
                    engine=inst.engine,
                )
                _orig_commit(self, nop, lazy_reg_writes)
            inst.sync_info = mybir.SyncInfo(
                on_wait=[waits[-1]], on_update=list(si.on_update)
            )
        return _orig_commit(self, inst, lazy_reg_writes)

    tile_mod.TileContext._commit_instruction = _commit_split

    def _patched_drain_and_barrier(self, tick_clock, wait_clock):
        gc = tick_clock.global_clock
        alloc = self.sems.allocated()
        for proc, sem in sorted(alloc.items()):
            tick = gc[proc]
            if tick > 0:
                mult = 16 if sem.name.startswith("DMA") else 1
                self.nc.sync.wait_ge(sem, tick * mult)
        self.nc.sync.drain()
        self.nc.all_engine_barrier()
        assert self.sems is not None
        popped = self.nc._tile_sem_poison_stack.pop()
        assert popped is self._sem_poison
        self.nc.clear_and_free_semaphores(list(self.sems.allocated().values()))

    tile_mod.TileContext._drain_and_barrier = _patched_drain_and_barrier


# ---------------------------------------------------------------------------
# Device program
# ---------------------------------------------------------------------------
def _build_nc():
    nc = bass.Bass()

    xft = nc.dram_tensor("xft", [128, 25], BF16, kind="ExternalInput")
    w1 = nc.dram_tensor("w1", [128, 25, 512], BF16, kind="ExternalInput")
    w2 = nc.dram_tensor("w2", [128, 5, 128], BF16, kind="ExternalInput")
    w3 = nc.dram_tensor("w3", [128, 2, 64], F32R, kind="ExternalInput")
    wd1 = nc.dram_tensor("wd1", [64, 128], F32R, kind="ExternalInput")
    wd1g = nc.dram_tensor("wd1g", [4, 128], F32R, kind="ExternalInput")
    gridt = nc.dram_tensor("gridt", [4, 64], F32R, kind="ExternalInput")
    wd2 = nc.dram_tensor("wd2", [128, 2, 512], BF16, kind="ExternalInput")
    wd3 = nc.dram_tensor("wd3", [128, 5, 768], BF16, kind="ExternalInput")
    s3tl = nc.dram_tensor("s3tl", [128, 8, 3], F32, kind="ExternalInput")
    onespad = nc.dram_tensor("onespad", [128, 1], F32R, kind="ExternalInput")

    outv = nc.dram_tensor("outv", [128, 9], F32, kind="ExternalOutput")

    dsc = nc.dram_tensor("dsc", [64, 2560], BF16)   # bounce for phiT gather

    with TileContext(nc) as tc:
        with tc.tile_pool(name="pers", bufs=1) as pers, \
             tc.tile_pool(name="wts", bufs=1) as wts, \
             tc.tile_pool(name="ps", bufs=4, space="PSUM") as psp:

            # psi/phi live at partitions 0:15 (PE row-group 0) with replicas
            # at partitions 64:79 (row-group 2) so the per-tile column-half
            # matmuls run concurrently on separate array tiles.
            psiT = pers.tile([79, 1024], BF16)
            phiT = pers.tile([79, 16384], BF16)
            colrun = pers.tile([128, 2, 1024], F16)
            rowstore = pers.tile([128, NT], F32)
            h1T = pers.tile([128, 5], BF16)
            h2T = pers.tile([128, 2], F32R)
            zrelu = pers.tile([64, 1], F32)
            zbT = pers.tile([64, 64], F32R)
            onesb = pers.tile([128, 64], F32R)
            onesbh = pers.tile([128, 128], BF16)
            h1d = pers.tile([64, 128], F32)
            h1dT = pers.tile([128, 128], BF16)
            h2d = pers.tile([64, 512], F32)
            h2dT = pers.tile([128, 4, 128], BF16)
            colT = pers.tile([128, 8, 128], F16)
            outt = pers.tile([128, 9], F32)

            with tc.tile_pool(name="tmp", bufs=1) as tmp:
                # ------- DMAs in consumption order ----
                # Each HWDGE ring runs at only ~100-160GB/s, so the big
                # weights are split across BOTH rings (w1/wd3 chunks
                # alternate) and the phi-pipeline DMAs queue behind a
                # balanced ~2.4MB on each ring instead of 4.9MB on one.
                s3t = tmp.tile([128, 8, 3], F32)
                nc.sync.dma_start(s3t[:], s3tl[:])
                xftt = tmp.tile([128, 25], BF16)
                nc.scalar.dma_start(xftt[:], xft[:])
                w1c = []
                for i, (k0, kn) in enumerate(W1CH):
                    w1ci = tmp.tile([128, kn, 512], BF16, tag=f"w1c{i}")
                    w1c.append(w1ci)
                    er = nc.sync if i % 2 == 0 else nc.scalar
                    er.dma_start(w1ci[:], w1[:, k0:k0 + kn, :])

                # ACT table load early on the scalar queue (Copy needs no
                # table; the first table user is the mm1 Relu at ~17us).
                warms = tmp.tile([1, 1], F32)
                nc.vector.memset(warms[:], 1.0)
                warmact = tmp.tile([1, 1], F32)
                nc.scalar.activation(warmact[:], warms[:], AFT.Relu)

                ident = wts.tile([128, 128], F32)
                make_identity(nc, ident[:])
                identh = wts.tile([128, 128], BF16)
                make_identity(nc, identh[:])
                identf = wts.tile([128, 128], F16)
                make_identity(nc, identf[:])

                # zero-pad the decoder stationaries once (full-height
                # matmuls stream 2 cols/cycle vs 1 for quadrant-tiled)
                nc.vector.memset(h1dT[:].rearrange("p f -> p f"), 0.0)
                nc.vector.memset(h2dT[:].rearrange("p a b -> p (a b)"), 0.0)
                nc.vector.memset(onesbh[:].rearrange("p f -> p f"), 1.0)
                # small weights + decoder weights
                w2t = wts.tile([128, 5, 128], BF16)
                nc.scalar.dma_start(w2t[:], w2[:])
                w3t = wts.tile([128, 2, 64], F32R)
                nc.scalar.dma_start(w3t[:], w3[:])
                wd1t = wts.tile([64, 128], F32R)
                nc.scalar.dma_start(wd1t[:], wd1[:])
                wd1gt = wts.tile([4, 128], F32R)
                nc.scalar.dma_start(wd1gt[:], wd1g[:])
                gridtt = wts.tile([4, 64], F32R)
                nc.scalar.dma_start(gridtt[:], gridt[:])
                onesp = wts.tile([128, 1], F32R)
                nc.scalar.dma_start(onesp[:], onespad[:])
                wd2t = wts.tile([128, 2, 512], BF16)
                nc.sync.dma_start(wd2t[:], wd2[:])
                wd3t = wts.tile([128, 5, 768], BF16)
                for k in range(5):
                    er = nc.sync if k % 2 == 0 else nc.scalar
                    er.dma_start(wd3t[:, k:k + 1, :], wd3[:, k:k + 1, :])

                # ---------------- psi (target lift) ----------------
                # stage k-layout: [hi(5), hi(5), lo(5)]; m = mt*128 + p
                stage = tmp.tile([128, 8, K], BF16)
                sq = tmp.tile([128, 8, 3], F32)
                nc.vector.tensor_tensor(sq[:], s3t[:], s3t[:], op=A.mult)
                s2t = tmp.tile([128, 8], F32)
                nc.vector.tensor_reduce(s2t[:], sq[:], axis=AX.X, op=A.add)
                m2 = tmp.tile([128, 8, 3], F32)
                nc.vector.tensor_scalar_mul(m2[:], s3t[:], -2.0)
                s2v = s2t[:].rearrange("p (t o) -> p t o", o=1)
                nc.vector.tensor_copy(stage[:, :, 0:3], m2[:])
                nc.vector.tensor_copy(stage[:, :, 5:8], stage[:, :, 0:3])
                nc.vector.memset(stage[:, :, 3:4], 1.0)
                nc.vector.memset(stage[:, :, 8:9], 1.0)
                nc.vector.tensor_copy(stage[:, :, 4:5], s2v)
                nc.vector.tensor_copy(stage[:, :, 9:10], stage[:, :, 4:5])
                m2hf = tmp.tile([128, 8, 3], F32)
                nc.vector.tensor_copy(m2hf[:], stage[:, :, 0:3])
                nc.vector.tensor_tensor(
                    stage[:, :, 10:13], m2[:], m2hf[:], op=A.subtract
                )
                nc.vector.memset(stage[:, :, 13:14], 0.0)
                s2hf = tmp.tile([128, 8], F32)
                nc.vector.tensor_copy(s2hf[:], stage[:, :, 4:5])
                nc.vector.tensor_tensor(
                    stage[:, :, 14:15], s2v,
                    s2hf[:].rearrange("p (t o) -> p t o", o=1), op=A.subtract,
                )
                for mt in range(8):
                    psm = psp.tile([K, 128], BF16, tag="ps")
                    nc.tensor.transpose(psm[:], stage[:, mt, :], identh[:])
                    nc.scalar.copy(psiT[0:K, mt * 128:(mt + 1) * 128], psm[:])
                # replicate psi to partitions 64:79 (PE row-group 2)
                nc.scalar.dma_start(psiT[64:64 + K, :], psiT[0:K, :])

                nc.vector.tensor_copy(
                    onesb[:], onesp[:].broadcast_to([128, 64]))

                # ---------------- encoder ----------------
                # mm1 in bf16: chunk kt goes to PE column group kt%3 (out
                # partitions 0/32/64) so three 512-col streams run
                # concurrently and mm1 finishes right behind the w1 DMA.
                # The three partial rows are summed by a tiny selector
                # matmul (junk partitions of y1c are zeroed by msel).
                y1p = psp.tile([1, 512], F32, tag="ps")
                for kt in range(25):
                    ci = next(i for i, (k0, kn) in enumerate(W1CH)
                              if k0 <= kt < k0 + kn)
                    k0, _ = W1CH[ci]
                    nc.tensor.matmul(
                        y1p[:],
                        xftt[:, kt:kt + 1],
                        w1c[ci][:, kt - k0, :],
                        start=(kt == 0),
                        stop=(kt == 24),
                    )
                h1sb = tmp.tile([1, 512], F32)
                nc.scalar.activation(h1sb[:], y1p[:], AFT.Relu)
                for mc in range(4):
                    tp1 = psp.tile([128, 1], F32, tag="ps")
                    nc.tensor.transpose(
                        tp1[:], h1sb[0:1, mc * 128:(mc + 1) * 128],
                        ident[0:1, 0:1],
                    )
                    if mc % 2 == 0:
                        nc.scalar.copy(h1T[:, mc:mc + 1], tp1[:])
                    else:
                        nc.vector.tensor_copy(h1T[:, mc:mc + 1], tp1[:])
                nc.vector.tensor_copy(h1T[:, 4:5], onesp[:])

                y2p = psp.tile([1, 128], F32, tag="ps")
                for kt in range(5):
                    nc.tensor.matmul(
                        y2p[:], h1T[:, kt:kt + 1], w2t[:, kt, :],
                        start=(kt == 0), stop=(kt == 4),
                    )
                h2sb = tmp.tile([1, 128], F32)
                nc.scalar.activation(h2sb[:], y2p[:], AFT.Relu)
                tp2 = psp.tile([128, 1], F32, tag="ps")
                nc.tensor.transpose(tp2[:], h2sb[:], ident[0:1, 0:1])
                nc.scalar.copy(h2T[:, 0:1], tp2[:])
                nc.vector.tensor_copy(h2T[:, 1:2], onesp[:])

                zp = psp.tile([1, 64], F32, tag="ps")
                for kt in range(2):
                    nc.tensor.matmul(
                        zp[:], h2T[:, kt:kt + 1], w3t[:, kt, :],
                        start=(kt == 0), stop=(kt == 1),
                    )
                zsb = tmp.tile([1, 64], F32)
                nc.scalar.activation(zsb[:], zp[:], AFT.Relu)
                tp3 = psp.tile([64, 1], F32, tag="ps")
                nc.tensor.transpose(tp3[:], zsb[:], ident[0:1, 0:1])
                nc.scalar.copy(zrelu[:], tp3[:])

                # ---------------- decoder ----------------
                nc.vector.tensor_copy(zbT[:], zrelu[:].broadcast_to([64, 64]))

                d1p = psp.tile([64, 128], F32, tag="ps")
                nc.tensor.matmul(d1p[:], zbT[:].bitcast(F32R),
                                 wd1t[:].bitcast(F32R), start=True, stop=False)
                nc.tensor.matmul(
                    d1p[:], gridtt[:].bitcast(F32R), wd1gt[:].bitcast(F32R),
                    start=False, stop=True
                )
                nc.scalar.activation(h1d[:], d1p[:], AFT.Relu)

                tr1p = psp.tile([128, 64], F32, tag="ps")
                nc.tensor.transpose(tr1p[:], h1d[:], ident[0:64, 0:64])
                nc.scalar.copy(h1dT[:, 0:64], tr1p[:])

                d2p = psp.tile([128, 512], F32, tag="ps")
                nc.tensor.matmul(
                    d2p[:], h1dT[:], wd2t[:, 0, :], start=True, stop=False
                )
                nc.tensor.matmul(
                    d2p[:], onesbh[:], wd2t[:, 1, :], start=False, stop=True
                )
                nc.scalar.activation(h2d[:], d2p[0:64, :], AFT.Relu)

                for kt in range(4):
                    trp = psp.tile([128, 64], F32, tag="ps")
                    nc.tensor.transpose(
                        trp[:], h2d[:, kt * 128:(kt + 1) * 128],
                        ident[0:64, 0:64],
                    )
                    nc.scalar.copy(h2dT[:, kt, 0:64], trp[:])

                d3p = psp.tile([128, 768], F32, tag="ps")
                for c0, w in ((0, 512), (512, 256)):
                    for kt in range(4):
                        nc.tensor.matmul(
                            d3p[:, c0:c0 + w], h2dT[:, kt, :],
                            wd3t[:, kt, c0:c0 + w],
                            start=(kt == 0), stop=False,
                        )
                    nc.tensor.matmul(
                        d3p[:, c0:c0 + w], onesbh[:], wd3t[:, 4, c0:c0 + w],
                        start=False, stop=True,
                    )

                # Lst static blocks first (off the critical path)
                Lst = tmp.tile([64, 2560], BF16)
                nc.vector.memset(Lst[:, 4 * 256:5 * 256], 1.0)
                nc.vector.memset(Lst[:, 9 * 256:10 * 256], 0.0)

                Y4 = tmp.tile([64, 4, 256], F32)
                nc.scalar.activation(Y4[:, 0, :], d3p[0:64, 0:768:3], AFT.Tanh)
                nc.scalar.activation(Y4[:, 1, :], d3p[0:64, 1:768:3], AFT.Tanh)
                nc.scalar.activation(Y4[:, 2, :], d3p[0:64, 2:768:3], AFT.Tanh)

                # ---------------- phi (generated lift) ----------------
                tmp2 = tmp.tile([64, 256], F32)
                nc.vector.tensor_tensor(
                    Y4[:, 3, :], Y4[:, 0, :], Y4[:, 0, :], op=A.mult)
                nc.vector.tensor_tensor(
                    tmp2[:], Y4[:, 1, :], Y4[:, 1, :], op=A.mult)
                nc.vector.tensor_tensor(
                    Y4[:, 3, :], Y4[:, 3, :], tmp2[:], op=A.add)
                nc.vector.tensor_tensor(
                    tmp2[:], Y4[:, 2, :], Y4[:, 2, :], op=A.mult)
                nc.vector.tensor_tensor(
                    Y4[:, 3, :], Y4[:, 3, :], tmp2[:], op=A.add)

                yflat = Y4[:].rearrange("g k j -> g (k j)")
                nc.vector.tensor_copy(Lst[:, 0:1024], yflat)
                hk4 = tmp.tile([64, 1024], F32)
                nc.vector.tensor_copy(hk4[:], Lst[:, 0:1024])
                nc.vector.tensor_tensor(
                    Lst[:, 5 * 256:9 * 256], yflat, hk4[:], op=A.subtract)

                # bounce Lst through DRAM in pieces, then gather the phi
                # rows per piece.  The first NDIRECT pieces fill the dup
                # rows 10:15 and the q64 replica 64:79 straight from DRAM
                # (parallel, one hop after the bounce-out); later pieces
                # use wide SBUF dup/replica copies (5+15 big descriptors).
                for pi, (g0, g1) in enumerate(OUTPIECES):
                    er = nc.sync if pi % 2 == 0 else nc.scalar
                    er.dma_start(dsc[g0:g1, :], Lst[g0:g1, :])
                src3 = dsc[:].rearrange("g (k j) -> k g j", k=10)
                dst3 = phiT[0:10, :].rearrange("k (g j) -> k g j", g=64)
                dstd = phiT[10:15, :].rearrange("k (g j) -> k g j", g=64)
                dstr = phiT[64:74, :].rearrange("k (g j) -> k g j", g=64)
                dstr2 = phiT[74:79, :].rearrange("k (g j) -> k g j", g=64)
                for pi, (g0, g1) in enumerate(GPIECES):
                    c0, c1 = g0 * 256, g1 * 256
                    er = nc.sync if pi % 2 == 0 else nc.scalar
                    er2 = nc.scalar if pi % 2 == 0 else nc.sync
                    er.dma_start(dst3[:, g0:g1, :], src3[:, g0:g1, :])
                    if pi < NDIRECT:
                        er2.dma_start(dstd[:, g0:g1, :], src3[0:5, g0:g1, :])
                        er2.dma_start(dstr[:, g0:g1, :], src3[:, g0:g1, :])
                        er.dma_start(dstr2[:, g0:g1, :], src3[0:5, g0:g1, :])
                    else:
                        er2.dma_start(phiT[10:15, c0:c1], phiT[0:5, c0:c1])
                        er.dma_start(phiT[64:79, c0:c1], phiT[0:15, c0:c1])

            # ---------------- distance phase ----------------
            # (separate pool scope so it reuses the closed tmp pool's SBUF)
            with tc.tile_pool(name="dist", bufs=2) as distp:
              for bi, (t0, tb) in enumerate(BATCHES):
                  bbt = distp.tile([128, 16, 1024], F16, tag="bb", bufs=2)
                  bb = bbt[:, 0:tb, :]
                  for i in range(tb):
                      t = t0 + i
                      ps = psp.tile([128, 1024], F32, tag="ps")
                      # two column-half matmuls on PE row-groups 0 and 2
                      # (concurrent; tile_position auto from base_partition)
                      nc.tensor.matmul(
                          ps[:, 0:512],
                          phiT[0:K, t * 128:(t + 1) * 128],
                          psiT[0:K, 0:512],
                          start=True, stop=True,
                      )
                      nc.tensor.matmul(
                          ps[:, 512:1024],
                          phiT[64:64 + K, t * 128:(t + 1) * 128],
                          psiT[64:64 + K, 512:1024],
                          start=True, stop=True,
                      )
                      nc.scalar.copy(bbt[:, i, :], ps[:])

                  # row-min fold chain (reads bb before the col tree
                  # overwrites it; both run on the DVE in program order)
                  f1t = distp.tile([128, 16, 512], F16, tag="f1", bufs=1)
                  nc.vector.tensor_tensor(
                      f1t[:, 0:tb, :], bb[:, :, 0:512], bb[:, :, 512:1024],
                      op=A.min
                  )
                  f2t = distp.tile([128, 16, 256], F16, tag="f2", bufs=1)
                  nc.vector.tensor_tensor(
                      f2t[:, 0:tb, :], f1t[:, 0:tb, 0:256],
                      f1t[:, 0:tb, 256:512], op=A.min
                  )
                  f3t = distp.tile([128, 16, 128], F16, tag="f3", bufs=1)
                  nc.vector.tensor_tensor(
                      f3t[:, 0:tb, :], f2t[:, 0:tb, 0:128],
                      f2t[:, 0:tb, 128:256], op=A.min
                  )
                  f4t = distp.tile([128, 16, 64], F16, tag="f4", bufs=1)
                  nc.vector.tensor_tensor(
                      f4t[:, 0:tb, :], f3t[:, 0:tb, 0:64],
                      f3t[:, 0:tb, 64:128], op=A.min
                  )
                  f5t = distp.tile([128, 16, 32], F16, tag="f5", bufs=1)
                  nc.vector.tensor_tensor(
                      f5t[:, 0:tb, :], f4t[:, 0:tb, 0:32],
                      f4t[:, 0:tb, 32:64], op=A.min
                  )
                  nc.vector.tensor_reduce(
                      rowstore[:, t0:t0 + tb], f5t[:, 0:tb, :],
                      axis=AX.X, op=A.min,
                  )

                  # column path: tree-min the batch in place down to 2
                  # lanes, then fold into colrun (same total element count
                  # as per-lane running mins, but only 2 persistent lanes)
                  h = tb // 2
                  while h >= 2:
                      nc.vector.tensor_tensor(
                          bbt[:, 0:h, :].rearrange("p t m -> p (t m)"),
                          bbt[:, 0:h, :].rearrange("p t m -> p (t m)"),
                          bbt[:, h:2 * h, :].rearrange("p t m -> p (t m)"),
                          op=A.min,
                      )
                      h //= 2
                  if bi == 0:
                      nc.vector.tensor_copy(
                          colrun[:].rearrange("p t m -> p (t m)"),
                          bbt[:, 0:2, :].rearrange("p t m -> p (t m)"),
                      )
                  else:
                      nc.vector.tensor_tensor(
                          colrun[:].rearrange("p t m -> p (t m)"),
                          colrun[:].rearrange("p t m -> p (t m)"),
                          bbt[:, 0:2, :].rearrange("p t m -> p (t m)"),
                          op=A.min,
                      )

              # ---------------- epilogue ----------------
              nc.vector.tensor_reduce(
                  outt[:, 8:9], rowstore[:], axis=AX.X, op=A.add
              )
              nc.vector.tensor_tensor(
                  colrun[:, 0, :], colrun[:, 0, :], colrun[:, 1, :], op=A.min,
              )
              for t in range(8):
                  trp2 = psp.tile([128, 128], F16, tag="ps")
                  nc.tensor.transpose(
                      trp2[:], colrun[:, 0, t * 128:(t + 1) * 128], identf[:]
                  )
                  nc.scalar.copy(colT[:, t, :], trp2[:])
              nc.vector.tensor_reduce(
                  outt[:, 0:8], colT[:], axis=AX.X, op=A.min
              )

              nc.sync.dma_start(outv[:], outt[:])

    return nc


_NC_CACHE = {}


def _get_nc():
    if "nc" not in _NC_CACHE:
        _NC_CACHE["nc"] = _build_nc()
    return _NC_CACHE["nc"]


def _fp22(a):
    """Truncate f32 mantissa to 13 bits (FP32r) so DMA'd data is pre-rounded."""
    b = np.ascontiguousarray(a, dtype=np.float32).view(np.uint32) & np.uint32(0xFFFFFC00)
    return b.view(np.float32)


def _bf16(a):
    return np.ascontiguousarray(a, dtype=np.float32).astype(ml_dtypes.bfloat16)


def _tiles(Wb, kt):
    """[K, N] -> [128, kt, N] partition-tiled, zero-padded."""
    K, N = Wb.shape
    pad = kt * 128 - K
    if pad:
        Wb = np.concatenate([Wb, np.zeros((pad, N), np.float32)], axis=0)
    return np.ascontiguousarray(Wb.reshape(kt, 128, N).transpose(1, 0, 2))


def prepare_in_maps(x, grid, We1, be1, We2, be2, We3, be3,
                    Wd1, bd1, Wd2, bd2, Wd3, bd3):
    f = lambda a: np.asarray(a, dtype=np.float32)
    x, grid = f(x), f(grid)
    We1, be1, We2, be2, We3, be3 = map(f, (We1, be1, We2, be2, We3, be3))
    Wd1, bd1, Wd2, bd2, Wd3, bd3 = map(f, (Wd1, bd1, Wd2, bd2, Wd3, bd3))

    w1h = _bf16(_tiles(np.vstack([We1, be1[None]]), 25))
    w2h = _bf16(_tiles(np.vstack([We2, be2[None]]), 5))
    w3h = _fp22(_tiles(np.vstack([We3, be3[None]]), 2))
    wd1h = _fp22(np.ascontiguousarray(Wd1[:64]))
    wd1gh = _fp22(np.vstack([Wd1[64:67], bd1[None]]))
    gridth = _fp22(np.vstack([grid.T, np.ones((1, G), np.float32)]))
    wd2h = _bf16(_tiles(np.vstack([Wd2, bd2[None]]), 2))
    wd3qh = [
        _bf16(_tiles(
            np.vstack([Wd3[:, 768 * q:768 * (q + 1)],
                       bd3[768 * q:768 * (q + 1)][None]]), 5
        ))
        for q in range(4)
    ]
    onespad = np.zeros((128, 1), np.float32)
    onespad[0, 0] = 1.0

    xfth = []
    s3h = []
    for b in range(B):
        xf_aug = np.zeros(3200, np.float32)
        xf_aug[:3072] = x[b].reshape(-1)
        xf_aug[3072] = 1.0
        xfth.append(_bf16(np.ascontiguousarray(xf_aug.reshape(25, 128).T)))
        # s3tl[p, mt, :] = x[b, mt*128 + p, :]
        s3h.append(np.ascontiguousarray(
            x[b].reshape(8, 128, 3).transpose(1, 0, 2)))

    in_maps = []
    for c in range(NCORES):
        b, q = c // 4, c % 4
        in_maps.append({
            "xft": xfth[b], "w1": w1h, "w2": w2h, "w3": w3h,
            "wd1": wd1h, "wd1g": wd1gh, "gridt": gridth,
            "wd2": wd2h, "wd3": wd3qh[q],
            "s3tl": s3h[b], "onespad": onespad,
        })
    return in_maps


def combine(results):
    loss = 0.0
    for c in range(NCORES):
        loss += float(results[c]["outv"][:, 8].astype(np.float64).sum())
    for b in range(B):
        parts = np.stack([results[c]["outv"][:, 0:8]
                          for c in range(4 * b, 4 * b + 4)])
        loss += float(parts.min(axis=0).astype(np.float64).sum())
    return np.float32(loss)


def kernel(x, grid, We1, be1, We2, be2, We3, be3,
           Wd1, bd1, Wd2, bd2, Wd3, bd3, **run_kwargs):
    nc = _get_nc()
    in_maps = prepare_in_maps(x, grid, We1, be1, We2, be2, We3, be3,
                              Wd1, bd1, Wd2, bd2, Wd3, bd3)
    res = run_bass_kernel_spmd(nc, in_maps, core_ids=list(range(NCORES)),
                               **run_kwargs)
    out = combine(res.results)
    kernel.last_results = res
    return out


# revision 16
# speedup vs baseline: 1.0141x; 1.0141x over previous
"""Trainium2 Bass kernel for nn_AutoEncoder3D (chamfer-loss autoencoder).

Strategy (8 NeuronCores, SPMD with per-core data):
  core c -> batch b = c // 4, quarter q = c % 4 of generated points.
  Each core: full encoder (bf16 matmuls), decoder for its quarter of the
  3072 output columns, then fused cdist+min over the [16384, 1024] chamfer
  block using a lifted-embedding bf16 hi/lo matmul on the PE, ACT
  PSUM->SBUF fp16 copies, and DVE min folds.  Host combines per-core
  row-min sums and col-min partials.

v5 deltas vs v4 (262us traced):
  - K=15 lift instead of 20: psi rows [hi, hi, lo], phi rows
    [hi, lo, hi-dup] -- drops only the negligible lo*lo term and halves
    the phi dup/replica SBUF traffic.
  - preamble: ACT table load first (no ident dependency), no mm1-gating
    of decoder weights (wd3 rides the SP ring behind w1, chunk-gated
    d3p), w1 tail chunks split small, decoder matmuls padded to full
    128-partition height (2 col/cycle instead of quadrant-tiled
    1 col/cycle: d3p stream 8.3us -> ~4us).
  - phi pipeline: DRAM bounce gathers only the 10 unique rows; the hi
    dup (10:15) and the q64 replica (64:79) are cheap wide SBUF->SBUF
    DMAs per piece; pieces sized (2,6,16,20,20) grid cells so the
    distance loop starts ~10us earlier and is never gather-starved.
  - colrun kept as 2 lanes; each 16-tile batch tree-mins its bb block
    in place (same DVE element count, one extra instruction) so the
    epilogue fold collapses from ~4.6us to one 1024-wide min.
  - epilogue: single lane fold + transposes + one reduce; tail batches
    end at 8 tiles.
"""

import numpy as np
import ml_dtypes

import concourse.bass as bass
import concourse.mybir as mybir
import concourse.tile as tile_mod
from concourse.bass_utils import run_bass_kernel_spmd
from concourse.masks import make_identity
from concourse.tile import TileContext

F32 = mybir.dt.float32
F32R = mybir.dt.float32r
F16 = mybir.dt.float16
BF16 = mybir.dt.bfloat16
A = mybir.AluOpType
AFT = mybir.ActivationFunctionType
AX = mybir.AxisListType

B = 2
G = 64
M = 1024
NCORES = 8
JQ = 256          # generated points per grid cell handled per core
NLOC = G * JQ     # 16384 generated points per core
NT = NLOC // 128  # 128 n-tiles
K = 15            # lift rows: [psi_hi, psi_hi, psi_lo] x [phi_hi, phi_lo, phi_hi]

# w1 k-chunk splits (tail kept small so mm1 finishes right after the DMA)
W1CH = [(0, 5), (5, 5), (10, 5), (15, 5), (20, 3), (23, 2)]

# phi piece schedule (grid-cell ranges).  Tiles of piece p unblock as its
# gather (+dup/replica) DMAs land; consumption (~1.34us/tile) is far slower
# than the gather so only piece 0 gates the loop start.  The first two
# pieces gather every phi row region directly from the DRAM bounce in
# parallel (one hop after the bounce-out); the rest use cheap wide SBUF
# copies for the dup/replica rows.
OUTPIECES = [(0, 2), (2, 8), (8, 24), (24, 64)]
GPIECES = [(0, 2), (2, 8), (8, 24), (24, 44), (44, 64)]
NDIRECT = 2

# distance-phase batches, aligned to piece tile boundaries
# pieces cover tiles [0,4), [4,16), [16,48), [48,88), [88,128)
BATCHES = [(0, 2), (2, 2), (4, 4), (8, 8),
           (16, 16), (32, 16), (48, 16), (64, 16), (80, 8),
           (88, 16), (104, 16), (120, 8)]


# ---------------------------------------------------------------------------
# Tile-framework patches: this walrus build allows at most ONE sync wait per
# instruction.  (a) split multi-wait instructions with preceding no-ops,
# (b) replace the context-exit drain (which carries one wait per live proc)
# with individual SP wait_ge instructions and a single exit barrier.
# ---------------------------------------------------------------------------
if not getattr(tile_mod, "_ae3d_wait_patch", False):
    tile_mod._ae3d_wait_patch = True

    _orig_commit = tile_mod.TileContext._commit_instruction

    def _commit_split(self, inst, lazy_reg_writes=True):
        si = getattr(inst, "sync_info", None)
        if si is not None and si.on_wait and len(si.on_wait) > 1:
            waits = list(si.on_wait)
            for w in waits[:-1]:
                nop = mybir.InstNoOp(
                    name=self.nc.get_next_instruction_name(),
                    sync_info=mybir.SyncInfo(on_wait=[w], on_update=[]),
                    bass_nofuse=True,
                    engine=inst.engine,
                )
                _orig_commit(self, nop, lazy_reg_writes)
            inst.sync_info = mybir.SyncInfo(
                on_wait=[waits[-1]], on_update=list(si.on_update)
            )
        return _orig_commit(self, inst, lazy_reg_writes)

    tile_mod.TileContext._commit_instruction = _commit_split

    def _patched_drain_and_barrier(self, tick_clock, wait_clock):
        gc = tick_clock.global_clock
        alloc = self.sems.allocated()
        for proc, sem in sorted(alloc.items()):
            tick = gc[proc]
            if tick > 0:
                mult = 16 if sem.name.startswith("DMA") else 1
                self.nc.sync.wait_ge(sem, tick * mult)
        self.nc.sync.drain()
        self.nc.all_engine_barrier()
        assert self.sems is not None
        popped = self.nc._tile_sem_poison_stack.pop()
        assert popped is self._sem_poison
        self.nc.clear_and_free_semaphores(list(self.sems.allocated().values()))

    tile_mod.TileContext._drain_and_barrier = _patched_drain_and_barrier


# ---------------------------------------------------------------------------
# Device program
# ---------------------------------------------------------------------------
def _build_nc():
    nc = bass.Bass()

    xft = nc.dram_tensor("xft", [128, 25], BF16, kind="ExternalInput")
    w1 = nc.dram_tensor("w1", [128, 25, 512], BF16, kind="ExternalInput")
    w2 = nc.dram_tensor("w2", [128, 5, 128], BF16, kind="ExternalInput")
    w3 = nc.dram_tensor("w3", [128, 2, 64], F32R, kind="ExternalInput")
    wd1 = nc.dram_tensor("wd1", [64, 128], F32R, kind="ExternalInput")
    wd1g = nc.dram_tensor("wd1g", [4, 128], F32R, kind="ExternalInput")
    gridt = nc.dram_tensor("gridt", [4, 64], F32R, kind="ExternalInput")
    wd2 = nc.dram_tensor("wd2", [128, 2, 512], BF16, kind="ExternalInput")
    wd3 = nc.dram_tensor("wd3", [128, 5, 768], BF16, kind="ExternalInput")
    s3tl = nc.dram_tensor("s3tl", [128, 8, 3], F32, kind="ExternalInput")
    onespad = nc.dram_tensor("onespad", [128, 1], F32R, kind="ExternalInput")

    outv = nc.dram_tensor("outv", [128, 9], F32, kind="ExternalOutput")

    dsc = nc.dram_tensor("dsc", [64, 2560], BF16)   # bounce for phiT gather

    with TileContext(nc) as tc:
        with tc.tile_pool(name="pers", bufs=1) as pers, \
             tc.tile_pool(name="wts", bufs=1) as wts, \
             tc.tile_pool(name="ps", bufs=4, space="PSUM") as psp:

            # psi/phi live at partitions 0:15 (PE row-group 0) with replicas
            # at partitions 64:79 (row-group 2) so the per-tile column-half
            # matmuls run concurrently on separate array tiles.
            psiT = pers.tile([79, 1024], BF16)
            phiT = pers.tile([79, 16384], BF16)
            colrun = pers.tile([128, 2, 1024], F16)
            rowstore = pers.tile([128, NT], F32)
            h1T = pers.tile([128, 5], BF16)
            h2T = pers.tile([128, 2], F32R)
            zrelu = pers.tile([64, 1], F32)
            zbT = pers.tile([64, 64], F32R)
            onesb = pers.tile([128, 64], F32R)
            onesbh = pers.tile([128, 128], BF16)
            h1d = pers.tile([64, 128], F32)
            h1dT = pers.tile([128, 128], BF16)
            h2d = pers.tile([64, 512], F32)
            h2dT = pers.tile([128, 4, 128], BF16)
            colT = pers.tile([128, 8, 128], F16)
            outt = pers.tile([128, 9], F32)

            with tc.tile_pool(name="tmp", bufs=1) as tmp:
                # ------- DMAs in consumption order ----
                # Each HWDGE ring runs at only ~100-160GB/s, so the big
                # weights are split across BOTH rings (w1/wd3 chunks
                # alternate) and the phi-pipeline DMAs queue behind a
                # balanced ~2.4MB on each ring instead of 4.9MB on one.
                s3t = tmp.tile([128, 8, 3], F32)
                nc.sync.dma_start(s3t[:], s3tl[:])
                xftt = tmp.tile([128, 25], BF16)
                nc.scalar.dma_start(xftt[:], xft[:])
                w1c = []
                for i, (k0, kn) in enumerate(W1CH):
                    w1ci = tmp.tile([128, kn, 512], BF16, tag=f"w1c{i}")
                    w1c.append(w1ci)
                    er = nc.sync if i % 2 == 0 else nc.scalar
                    er.dma_start(w1ci[:], w1[:, k0:k0 + kn, :])

                # ACT table load early on the scalar queue (Copy needs no
                # table; the first table user is the mm1 Relu at ~17us).
                warms = tmp.tile([1, 1], F32)
                nc.vector.memset(warms[:], 1.0)
                warmact = tmp.tile([1, 1], F32)
                nc.scalar.activation(warmact[:], warms[:], AFT.Relu)

                ident = wts.tile([128, 128], F32)
                make_identity(nc, ident[:])
                identh = wts.tile([128, 128], BF16)
                make_identity(nc, identh[:])
                identf = wts.tile([128, 128], F16)
                make_identity(nc, identf[:])

                # zero-pad the decoder stationaries once (full-height
                # matmuls stream 2 cols/cycle vs 1 for quadrant-tiled)
                nc.vector.memset(h1dT[:].rearrange("p f -> p f"), 0.0)
                nc.vector.memset(h2dT[:].rearrange("p a b -> p (a b)"), 0.0)
                nc.vector.memset(onesbh[:].rearrange("p f -> p f"), 1.0)
                # small weights + decoder weights
                w2t = wts.tile([128, 5, 128], BF16)
                nc.scalar.dma_start(w2t[:], w2[:])
                w3t = wts.tile([128, 2, 64], F32R)
                nc.scalar.dma_start(w3t[:], w3[:])
                wd1t = wts.tile([64, 128], F32R)
                nc.scalar.dma_start(wd1t[:], wd1[:])
                wd1gt = wts.tile([4, 128], F32R)
                nc.scalar.dma_start(wd1gt[:], wd1g[:])
                gridtt = wts.tile([4, 64], F32R)
                nc.scalar.dma_start(gridtt[:], gridt[:])
                onesp = wts.tile([128, 1], F32R)
                nc.scalar.dma_start(onesp[:], onespad[:])

                # ---------------- psi (target lift) ----------------
                # stage k-layout: [hi(5), hi(5), lo(5)]; m = mt*128 + p
                stage = tmp.tile([128, 8, K], BF16)
                sq = tmp.tile([128, 8, 3], F32)
                nc.vector.tensor_tensor(sq[:], s3t[:], s3t[:], op=A.mult)
                s2t = tmp.tile([128, 8], F32)
                nc.vector.tensor_reduce(s2t[:], sq[:], axis=AX.X, op=A.add)
                m2 = tmp.tile([128, 8, 3], F32)
                nc.vector.tensor_scalar_mul(m2[:], s3t[:], -2.0)
                s2v = s2t[:].rearrange("p (t o) -> p t o", o=1)
                nc.vector.tensor_copy(stage[:, :, 0:3], m2[:])
                nc.vector.tensor_copy(stage[:, :, 5:8], stage[:, :, 0:3])
                nc.vector.memset(stage[:, :, 3:4], 1.0)
                nc.vector.memset(stage[:, :, 8:9], 1.0)
                nc.vector.tensor_copy(stage[:, :, 4:5], s2v)
                nc.vector.tensor_copy(stage[:, :, 9:10], stage[:, :, 4:5])
                m2hf = tmp.tile([128, 8, 3], F32)
                nc.vector.tensor_copy(m2hf[:], stage[:, :, 0:3])
                nc.vector.tensor_tensor(
                    stage[:, :, 10:13], m2[:], m2hf[:], op=A.subtract
                )
                nc.vector.memset(stage[:, :, 13:14], 0.0)
                s2hf = tmp.tile([128, 8], F32)
                nc.vector.tensor_copy(s2hf[:], stage[:, :, 4:5])
                nc.vector.tensor_tensor(
                    stage[:, :, 14:15], s2v,
                    s2hf[:].rearrange("p (t o) -> p t o", o=1), op=A.subtract,
                )
                for mt in range(8):
                    psm = psp.tile([K, 128], BF16, tag="ps")
                    nc.tensor.transpose(psm[:], stage[:, mt, :], identh[:])
                    nc.scalar.copy(psiT[0:K, mt * 128:(mt + 1) * 128], psm[:])
                # replicate psi to partitions 64:79 (PE row-group 2)
                nc.scalar.dma_start(psiT[64:64 + K, :], psiT[0:K, :])

                # decoder weights issued after the psi section so the ACT
                # engine stream isn't blocked ahead of the psi copies
                wd2t = wts.tile([128, 2, 512], BF16)
                nc.sync.dma_start(wd2t[:], wd2[:])
                wd3t = wts.tile([128, 5, 768], BF16)
                for k in range(5):
                    er = nc.sync if k % 2 == 0 else nc.scalar
                    er.dma_start(wd3t[:, k:k + 1, :], wd3[:, k:k + 1, :])

                nc.vector.tensor_copy(
                    onesb[:], onesp[:].broadcast_to([128, 64]))

                # ---------------- encoder ----------------
                # mm1 in bf16: chunk kt goes to PE column group kt%3 (out
                # partitions 0/32/64) so three 512-col streams run
                # concurrently and mm1 finishes right behind the w1 DMA.
                # The three partial rows are summed by a tiny selector
                # matmul (junk partitions of y1c are zeroed by msel).
                y1p = psp.tile([1, 512], F32, tag="ps")
                for kt in range(25):
                    ci = next(i for i, (k0, kn) in enumerate(W1CH)
                              if k0 <= kt < k0 + kn)
                    k0, _ = W1CH[ci]
                    nc.tensor.matmul(
                        y1p[:],
                        xftt[:, kt:kt + 1],
                        w1c[ci][:, kt - k0, :],
                        start=(kt == 0),
                        stop=(kt == 24),
                    )
                h1sb = tmp.tile([1, 512], F32)
                nc.scalar.activation(h1sb[:], y1p[:], AFT.Relu)
                for mc in range(4):
                    tp1 = psp.tile([128, 1], F32, tag="ps")
                    nc.tensor.transpose(
                        tp1[:], h1sb[0:1, mc * 128:(mc + 1) * 128],
                        ident[0:1, 0:1],
                    )
                    if mc % 2 == 0:
                        nc.scalar.copy(h1T[:, mc:mc + 1], tp1[:])
                    else:
                        nc.vector.tensor_copy(h1T[:, mc:mc + 1], tp1[:])
                nc.vector.tensor_copy(h1T[:, 4:5], onesp[:])

                y2p = psp.tile([1, 128], F32, tag="ps")
                for kt in range(5):
                    nc.tensor.matmul(
                        y2p[:], h1T[:, kt:kt + 1], w2t[:, kt, :],
                        start=(kt == 0), stop=(kt == 4),
                    )
                h2sb = tmp.tile([1, 128], F32)
                nc.scalar.activation(h2sb[:], y2p[:], AFT.Relu)
                tp2 = psp.tile([128, 1], F32, tag="ps")
                nc.tensor.transpose(tp2[:], h2sb[:], ident[0:1, 0:1])
                nc.scalar.copy(h2T[:, 0:1], tp2[:])
                nc.vector.tensor_copy(h2T[:, 1:2], onesp[:])

                zp = psp.tile([1, 64], F32, tag="ps")
                for kt in range(2):
                    nc.tensor.matmul(
                        zp[:], h2T[:, kt:kt + 1], w3t[:, kt, :],
                        start=(kt == 0), stop=(kt == 1),
                    )
                zsb = tmp.tile([1, 64], F32)
                nc.scalar.activation(zsb[:], zp[:], AFT.Relu)
                tp3 = psp.tile([64, 1], F32, tag="ps")
                nc.tensor.transpose(tp3[:], zsb[:], ident[0:1, 0:1])
                nc.scalar.copy(zrelu[:], tp3[:])

                # ---------------- decoder ----------------
                nc.vector.tensor_copy(zbT[:], zrelu[:].broadcast_to([64, 64]))

                d1p = psp.tile([64, 128], F32, tag="ps")
                nc.tensor.matmul(d1p[:], zbT[:].bitcast(F32R),
                                 wd1t[:].bitcast(F32R), start=True, stop=False)
                nc.tensor.matmul(
                    d1p[:], gridtt[:].bitcast(F32R), wd1gt[:].bitcast(F32R),
                    start=False, stop=True
                )
                nc.scalar.activation(h1d[:], d1p[:], AFT.Relu)

                tr1p = psp.tile([128, 64], F32, tag="ps")
                nc.tensor.transpose(tr1p[:], h1d[:], ident[0:64, 0:64])
                nc.scalar.copy(h1dT[:, 0:64], tr1p[:])

                d2p = psp.tile([128, 512], F32, tag="ps")
                nc.tensor.matmul(
                    d2p[:], h1dT[:], wd2t[:, 0, :], start=True, stop=False
                )
                nc.tensor.matmul(
                    d2p[:], onesbh[:], wd2t[:, 1, :], start=False, stop=True
                )
                nc.scalar.activation(h2d[:], d2p[0:64, :], AFT.Relu)

                for kt in range(4):
                    trp = psp.tile([128, 64], F32, tag="ps")
                    nc.tensor.transpose(
                        trp[:], h2d[:, kt * 128:(kt + 1) * 128],
                        ident[0:64, 0:64],
                    )
                    nc.scalar.copy(h2dT[:, kt, 0:64], trp[:])

                d3p = psp.tile([128, 768], F32, tag="ps")
                for c0, w in ((0, 512), (512, 256)):
                    for kt in range(4):
                        nc.tensor.matmul(
                            d3p[:, c0:c0 + w], h2dT[:, kt, :],
                            wd3t[:, kt, c0:c0 + w],
                            start=(kt == 0), stop=False,
                        )
                    nc.tensor.matmul(
                        d3p[:, c0:c0 + w], onesbh[:], wd3t[:, 4, c0:c0 + w],
                        start=False, stop=True,
                    )

                # Lst static blocks first (off the critical path)
                Lst = tmp.tile([64, 2560], BF16)
                nc.vector.memset(Lst[:, 4 * 256:5 * 256], 1.0)
                nc.vector.memset(Lst[:, 9 * 256:10 * 256], 0.0)

                Y4 = tmp.tile([64, 4, 256], F32)
                nc.scalar.activation(Y4[:, 0, :], d3p[0:64, 0:768:3], AFT.Tanh)
                nc.scalar.activation(Y4[:, 1, :], d3p[0:64, 1:768:3], AFT.Tanh)
                nc.scalar.activation(Y4[:, 2, :], d3p[0:64, 2:768:3], AFT.Tanh)

                # ---------------- phi (generated lift) ----------------
                tmp2 = tmp.tile([64, 256], F32)
                nc.vector.tensor_tensor(
                    Y4[:, 3, :], Y4[:, 0, :], Y4[:, 0, :], op=A.mult)
                nc.vector.tensor_tensor(
                    tmp2[:], Y4[:, 1, :], Y4[:, 1, :], op=A.mult)
                nc.vector.tensor_tensor(
                    Y4[:, 3, :], Y4[:, 3, :], tmp2[:], op=A.add)
                nc.vector.tensor_tensor(
                    tmp2[:], Y4[:, 2, :], Y4[:, 2, :], op=A.mult)
                nc.vector.tensor_tensor(
                    Y4[:, 3, :], Y4[:, 3, :], tmp2[:], op=A.add)

                yflat = Y4[:].rearrange("g k j -> g (k j)")
                nc.vector.tensor_copy(Lst[:, 0:1024], yflat)
                hk4 = tmp.tile([64, 1024], F32)
                nc.vector.tensor_copy(hk4[:], Lst[:, 0:1024])
                nc.vector.tensor_tensor(
                    Lst[:, 5 * 256:9 * 256], yflat, hk4[:], op=A.subtract)

                # bounce Lst through DRAM in pieces, then gather the phi
                # rows per piece.  The first NDIRECT pieces fill the dup
                # rows 10:15 and the q64 replica 64:79 straight from DRAM
                # (parallel, one hop after the bounce-out); later pieces
                # use wide SBUF dup/replica copies (5+15 big descriptors).
                for pi, (g0, g1) in enumerate(OUTPIECES):
                    er = nc.sync if pi % 2 == 0 else nc.scalar
                    er.dma_start(dsc[g0:g1, :], Lst[g0:g1, :])
                src3 = dsc[:].rearrange("g (k j) -> k g j", k=10)
                dst3 = phiT[0:10, :].rearrange("k (g j) -> k g j", g=64)
                dstd = phiT[10:15, :].rearrange("k (g j) -> k g j", g=64)
                dstr = phiT[64:74, :].rearrange("k (g j) -> k g j", g=64)
                dstr2 = phiT[74:79, :].rearrange("k (g j) -> k g j", g=64)
                for pi, (g0, g1) in enumerate(GPIECES):
                    c0, c1 = g0 * 256, g1 * 256
                    er = nc.sync if pi % 2 == 0 else nc.scalar
                    er2 = nc.scalar if pi % 2 == 0 else nc.sync
                    er.dma_start(dst3[:, g0:g1, :], src3[:, g0:g1, :])
                    if pi < NDIRECT:
                        er2.dma_start(dstd[:, g0:g1, :], src3[0:5, g0:g1, :])
                        er2.dma_start(dstr[:, g0:g1, :], src3[:, g0:g1, :])
                        er.dma_start(dstr2[:, g0:g1, :], src3[0:5, g0:g1, :])
                    else:
                        er2.dma_start(phiT[10:15, c0:c1], phiT[0:5, c0:c1])
                        er.dma_start(phiT[64:79, c0:c1], phiT[0:15, c0:c1])

            # ---------------- distance phase ----------------
            # (separate pool scope so it reuses the closed tmp pool's SBUF)
            with tc.tile_pool(name="dist", bufs=2) as distp:
              for bi, (t0, tb) in enumerate(BATCHES):
                  bbt = distp.tile([128, 16, 1024], F16, tag="bb", bufs=2)
                  bb = bbt[:, 0:tb, :]
                  for i in range(tb):
                      t = t0 + i
                      ps = psp.tile([128, 1024], F32, tag="ps")
                      # two column-half matmuls on PE row-groups 0 and 2
                      # (concurrent; tile_position auto from base_partition)
                      nc.tensor.matmul(
                          ps[:, 0:512],
                          phiT[0:K, t * 128:(t + 1) * 128],
                          psiT[0:K, 0:512],
                          start=True, stop=True,
                      )
                      nc.tensor.matmul(
                          ps[:, 512:1024],
                          phiT[64:64 + K, t * 128:(t + 1) * 128],
                          psiT[64:64 + K, 512:1024],
                          start=True, stop=True,
                      )
                      nc.scalar.copy(bbt[:, i, :], ps[:])

                  # row-min fold chain (reads bb before the col tree
                  # overwrites it; both run on the DVE in program order)
                  f1t = distp.tile([128, 16, 512], F16, tag="f1", bufs=1)
                  nc.vector.tensor_tensor(
                      f1t[:, 0:tb, :], bb[:, :, 0:512], bb[:, :, 512:1024],
                      op=A.min
                  )
                  f2t = distp.tile([128, 16, 256], F16, tag="f2", bufs=1)
                  nc.vector.tensor_tensor(
                      f2t[:, 0:tb, :], f1t[:, 0:tb, 0:256],
                      f1t[:, 0:tb, 256:512], op=A.min
                  )
                  f3t = distp.tile([128, 16, 128], F16, tag="f3", bufs=1)
                  nc.vector.tensor_tensor(
                      f3t[:, 0:tb, :], f2t[:, 0:tb, 0:128],
                      f2t[:, 0:tb, 128:256], op=A.min
                  )
                  f4t = distp.tile([128, 16, 64], F16, tag="f4", bufs=1)
                  nc.vector.tensor_tensor(
                      f4t[:, 0:tb, :], f3t[:, 0:tb, 0:64],
                      f3t[:, 0:tb, 64:128], op=A.min
                  )
                  f5t = distp.tile([128, 16, 32], F16, tag="f5", bufs=1)
                  nc.vector.tensor_tensor(
                      f5t[:, 0:tb, :], f4t[:, 0:tb, 0:32],
                      f4t[:, 0:tb, 32:64], op=A.min
                  )
                  nc.vector.tensor_reduce(
                      rowstore[:, t0:t0 + tb], f5t[:, 0:tb, :],
                      axis=AX.X, op=A.min,
                  )

                  # column path: tree-min the batch in place down to 2
                  # lanes, then fold into colrun (same total element count
                  # as per-lane running mins, but only 2 persistent lanes)
                  h = tb // 2
                  while h >= 2:
                      nc.vector.tensor_tensor(
                          bbt[:, 0:h, :].rearrange("p t m -> p (t m)"),
                          bbt[:, 0:h, :].rearrange("p t m -> p (t m)"),
                          bbt[:, h:2 * h, :].rearrange("p t m -> p (t m)"),
                          op=A.min,
                      )
                      h //= 2
                  if bi == 0:
                      nc.vector.tensor_copy(
                          colrun[:].rearrange("p t m -> p (t m)"),
                          bbt[:, 0:2, :].rearrange("p t m -> p (t m)"),
                      )
                  else:
                      nc.vector.tensor_tensor(
                          colrun[:].rearrange("p t m -> p (t m)"),
                          colrun[:].rearrange("p t m -> p (t m)"),
                          bbt[:, 0:2, :].rearrange("p t m -> p (t m)"),
                          op=A.min,
                      )

              # ---------------- epilogue ----------------
              nc.vector.tensor_reduce(
                  outt[:, 8:9], rowstore[:], axis=AX.X, op=A.add
              )
              nc.vector.tensor_tensor(
                  colrun[:, 0, :], colrun[:, 0, :], colrun[:, 1, :], op=A.min,
              )
              for t in range(8):
                  trp2 = psp.tile([128, 128], F16, tag="ps")
                  nc.tensor.transpose(
                      trp2[:], colrun[:, 0, t * 128:(t + 1) * 128], identf[:]
                  )
                  nc.scalar.copy(colT[:, t, :], trp2[:])
              nc.vector.tensor_reduce(
                  outt[:, 0:8], colT[:], axis=AX.X, op=A.min
              )

              nc.sync.dma_start(outv[:], outt[:])

    return nc


_NC_CACHE = {}


def _get_nc():
    if "nc" not in _NC_CACHE:
        _NC_CACHE["nc"] = _build_nc()
    return _NC_CACHE["nc"]


def _fp22(a):
    """Truncate f32 mantissa to 13 bits (FP32r) so DMA'd data is pre-rounded."""
    b = np.ascontiguousarray(a, dtype=np.float32).view(np.uint32) & np.uint32(0xFFFFFC00)
    return b.view(np.float32)


def _bf16(a):
    return np.ascontiguousarray(a, dtype=np.float32).astype(ml_dtypes.bfloat16)


def _tiles(Wb, kt):
    """[K, N] -> [128, kt, N] partition-tiled, zero-padded."""
    K, N = Wb.shape
    pad = kt * 128 - K
    if pad:
        Wb = np.concatenate([Wb, np.zeros((pad, N), np.float32)], axis=0)
    return np.ascontiguousarray(Wb.reshape(kt, 128, N).transpose(1, 0, 2))


def prepare_in_maps(x, grid, We1, be1, We2, be2, We3, be3,
                    Wd1, bd1, Wd2, bd2, Wd3, bd3):
    f = lambda a: np.asarray(a, dtype=np.float32)
    x, grid = f(x), f(grid)
    We1, be1, We2, be2, We3, be3 = map(f, (We1, be1, We2, be2, We3, be3))
    Wd1, bd1, Wd2, bd2, Wd3, bd3 = map(f, (Wd1, bd1, Wd2, bd2, Wd3, bd3))

    w1h = _bf16(_tiles(np.vstack([We1, be1[None]]), 25))
    w2h = _bf16(_tiles(np.vstack([We2, be2[None]]), 5))
    w3h = _fp22(_tiles(np.vstack([We3, be3[None]]), 2))
    wd1h = _fp22(np.ascontiguousarray(Wd1[:64]))
    wd1gh = _fp22(np.vstack([Wd1[64:67], bd1[None]]))
    gridth = _fp22(np.vstack([grid.T, np.ones((1, G), np.float32)]))
    wd2h = _bf16(_tiles(np.vstack([Wd2, bd2[None]]), 2))
    wd3qh = [
        _bf16(_tiles(
            np.vstack([Wd3[:, 768 * q:768 * (q + 1)],
                       bd3[768 * q:768 * (q + 1)][None]]), 5
        ))
        for q in range(4)
    ]
    onespad = np.zeros((128, 1), np.float32)
    onespad[0, 0] = 1.0

    xfth = []
    s3h = []
    for b in range(B):
        xf_aug = np.zeros(3200, np.float32)
        xf_aug[:3072] = x[b].reshape(-1)
        xf_aug[3072] = 1.0
        xfth.append(_bf16(np.ascontiguousarray(xf_aug.reshape(25, 128).T)))
        # s3tl[p, mt, :] = x[b, mt*128 + p, :]
        s3h.append(np.ascontiguousarray(
            x[b].reshape(8, 128, 3).transpose(1, 0, 2)))

    in_maps = []
    for c in range(NCORES):
        b, q = c // 4, c % 4
        in_maps.append({
            "xft": xfth[b], "w1": w1h, "w2": w2h, "w3": w3h,
            "wd1": wd1h, "wd1g": wd1gh, "gridt": gridth,
            "wd2": wd2h, "wd3": wd3qh[q],
            "s3tl": s3h[b], "onespad": onespad,
        })
    return in_maps


def combine(results):
    loss = 0.0
    for c in range(NCORES):
        loss += float(results[c]["outv"][:, 8].astype(np.float64).sum())
    for b in range(B):
        parts = np.stack([results[c]["outv"][:, 0:8]
                          for c in range(4 * b, 4 * b + 4)])
        loss += float(parts.min(axis=0).astype(np.float64).sum())
    return np.float32(loss)


def kernel(x, grid, We1, be1, We2, be2, We3, be3,
           Wd1, bd1, Wd2, bd2, Wd3, bd3, **run_kwargs):
    nc = _get_nc()
    in_maps = prepare_in_maps(x, grid, We1, be1, We2, be2, We3, be3,
                              Wd1, bd1, Wd2, bd2, Wd3, bd3)
    res = run_bass_kernel_spmd(nc, in_maps, core_ids=list(range(NCORES)),
                               **run_kwargs)
    out = combine(res.results)
    kernel.last_results = res
    return out
